# revision 1
# baseline (speedup 1.0000x reference)
"""Trainium2 Bass kernel for nn_CustomConv2d: 3x3 conv, stride 1, pad 1.

Full shapes: x (32,128,56,56) f32, weight (256,128,3,3) f32, bias (256,) f32.
Output: (32,256,56,56) f32.

Strategy: data-parallel over batch (8 cores x 4 images). Per image the conv is
9 accumulating PE matmuls per output tile: contraction dim = Cin = 128 (exactly
the PE array), stationary = weight tap (Cin x Cout_half), moving = shifted
window of the zero-padded input, free dim = 8 output rows x 56 cols = 448.
Inputs are streamed as float32r (full-rate fp32 PE mode, ~1e-4 rel err).
Input images stream in row-slabs (critical-first DMA order, few descriptors)
and outputs store per pair of 8-row chunks so DMA overlaps compute; the
PSUM->SBUF bias-copy alternates between ACT and DVE; dep-free warmup matmuls
cover the initial DMA wait and bring the PE clock to full rate.
"""

import numpy as np

import concourse.bass as bass
import concourse.mybir as mybir
import concourse.tile as tile
from concourse import bacc
from concourse.bass_utils import run_bass_kernel_spmd

N_CORES = 8
B = 32
B_LOC = B // N_CORES  # 4
CIN = 128
COUT = 256
H = W = 56
HP = WP = 58  # padded
RCH = 8  # output rows per matmul chunk
NCH = H // RCH  # 7

_NC_CACHE = None
LAST_RESULTS = None  # stashed BassKernelResults for test harness introspection


def _build(reps: int = 1) -> bass.Bass:
    f32 = mybir.dt.float32
    f32r = mybir.dt.float32r
    nc = bacc.Bacc(None, target_bir_lowering=False)
    x_d = nc.dram_tensor("x", [B_LOC, CIN, HP * WP], f32r, kind="ExternalInput")
    wt_d = nc.dram_tensor("wt", [CIN, 9 * COUT], f32r, kind="ExternalInput")
    b_d = nc.dram_tensor("b", [2, 128], f32, kind="ExternalInput")
    y_d = nc.dram_tensor("y", [B_LOC, COUT, H * W], f32, kind="ExternalOutput")

    wt3 = wt_d[:].rearrange("p (t o) -> p t o", t=9)

    from contextlib import ExitStack, nullcontext

    with tile.TileContext(nc) as tc, ExitStack() as es:
        cpool = es.enter_context(tc.tile_pool(name="const", bufs=1))
        xpool = es.enter_context(tc.tile_pool(name="xp", bufs=B_LOC))
        opool = es.enter_context(tc.tile_pool(name="out", bufs=6))
        pspool = es.enter_context(tc.tile_pool(name="ps", bufs=7, space="PSUM"))
        with tc.For_i(0, reps, 1) if reps > 1 else nullcontext():
            wtile = cpool.tile([CIN, 9, COUT], f32r)
            xpads = [
                xpool.tile([CIN, HP, WP], f32r, tag="xpad", name=f"xpad{i}")
                for i in range(B_LOC)
            ]

            # PE warmup: dep-free dummy matmuls fill the initial DMA wait and
            # bring the PE clock (HAM) to full rate before the real work.
            wsrc0 = cpool.tile([128, 64], f32)
            nc.vector.memset(wsrc0[:], 0.0)
            wsrc = cpool.tile([128, 64], f32r)
            nc.vector.tensor_copy(wsrc[:], wsrc0[:])
            wps = pspool.tile([64, 64], f32, tag="warmps", bufs=1)
            for _ in range(40):
                nc.tensor.matmul(wps[:], wsrc[:], wsrc[:], start=True, stop=True)

            # DMA issue order = criticality, fewest possible descriptors on
            # the critical path (each DMA pays ~0.6us serial queue time):
            # lead slab + tap0 weights first, then the rest in big chunks.
            xsrc0 = x_d[0].rearrange("p (h w) -> p h w", h=HP)
            nc.sync.dma_start(xpads[0][:, 0:10, :], xsrc0[:, 0:10, :])
            nc.sync.dma_start(wtile[:, 0, 0:128], wt3[:, 0, 0:128])
            nc.sync.dma_start(wtile[:, 1:9, 0:128], wt3[:, 1:9, 0:128])
            nc.sync.dma_start(xpads[0][:, 10:34, :], xsrc0[:, 10:34, :])
            nc.sync.dma_start(xpads[0][:, 34:58, :], xsrc0[:, 34:58, :])
            nc.sync.dma_start(wtile[:, :, 128:256], wt3[:, :, 128:256])
            btile = cpool.tile([128, 2], f32)
            nc.sync.dma_start(btile[:], b_d[:].rearrange("t p -> p t"))
            for b in range(1, B_LOC):
                xsrc = x_d[b].rearrange("p (h w) -> p h w", h=HP)
                nc.sync.dma_start(xpads[b][:, 0:29, :], xsrc[:, 0:29, :])
                nc.sync.dma_start(xpads[b][:, 29:58, :], xsrc[:, 29:58, :])

            for b in range(B_LOC):
                xpad = xpads[b]
                for t in range(2):
                    last_bt = (b == B_LOC - 1) and (t == 1)
                    # final output tile: singleton stores so the tail transfer
                    # is minimal; elsewhere pairs to halve descriptor count
                    groups = (
                        [[r] for r in range(NCH)]
                        if last_bt
                        else [[0, 1], [2, 3], [4, 5], [6]]
                    )
                    for rr in groups:
                        ochunk = opool.tile(
                            [128, len(rr), RCH * W], f32, tag="ochunk"
                        )
                        for i, r in enumerate(rr):
                            ps = pspool.tile([128, RCH, W], f32)
                            for tap in range(9):
                                ky, kx = divmod(tap, 3)
                                rhs = xpad[
                                    :, r * RCH + ky : r * RCH + ky + RCH, kx : kx + W
                                ]
                                lhsT = wtile[:, tap, t * 128 : (t + 1) * 128]
                                nc.tensor.matmul(
                                    ps[:], lhsT, rhs, start=(tap == 0), stop=(tap == 8)
                                )
                            psv = ps[:].rearrange("p h w -> p (h w)")
                            if r % 2 == 0:
                                nc.scalar.activation(
                                    ochunk[:, i, :],
                                    psv,
                                    mybir.ActivationFunctionType.Identity,
                                    bias=btile[:, t : t + 1],
                                )
                            else:
                                nc.vector.tensor_scalar_add(
                                    ochunk[:, i, :], psv, btile[:, t : t + 1]
                                )
                        nc.sync.dma_start(
                            y_d[
                                b,
                                t * 128 : (t + 1) * 128,
                                rr[0] * RCH * W : (rr[-1] + 1) * RCH * W,
                            ],
                            ochunk[:].rearrange("p c n -> p (c n)"),
                        )
    nc.finalize()
    return nc


def kernel(x, weight, bias, approximate):
    """Full (unsharded) conv2d. `approximate` only selects the HW approximation
    level in the original module; the exact-math output is independent of it."""
    global _NC_CACHE, LAST_RESULTS
    x = np.ascontiguousarray(x, dtype=np.float32)
    weight = np.ascontiguousarray(weight, dtype=np.float32)
    bias = np.ascontiguousarray(bias, dtype=np.float32)

    # zero-pad spatially on the host; shard batch across cores
    xp = np.zeros((B, CIN, HP, WP), np.float32)
    xp[:, :, 1 : H + 1, 1 : W + 1] = x
    xp = xp.reshape(B, CIN, HP * WP)
    wt = np.ascontiguousarray(weight.transpose(1, 2, 3, 0)).reshape(CIN, 9 * COUT)
    b2 = bias.reshape(2, 128)

    if _NC_CACHE is None:
        _NC_CACHE = _build()
    nc = _NC_CACHE

    in_maps = [
        {"x": xp[c * B_LOC : (c + 1) * B_LOC], "wt": wt, "b": b2}
        for c in range(N_CORES)
    ]
    try:
        res = run_bass_kernel_spmd(nc, in_maps, core_ids=list(range(N_CORES)))
    except Exception:
        # transient device-acquisition races (NRT_EXEC_UNIT_UNRECOVERABLE on
        # first touch after a prior process teardown) recover on retry
        import time as _time

        _time.sleep(5.0)
        res = run_bass_kernel_spmd(nc, in_maps, core_ids=list(range(N_CORES)))
    LAST_RESULTS = res
    out = np.concatenate([r["y"] for r in res.results], axis=0)
    return out.reshape(B, COUT, H, W)

